# revision 20
# baseline (speedup 1.0000x reference)
"""2-layer GCN (gnn_message_passing) on 8 Trainium2 NeuronCores.

Strategy (v3 - aggregate-first/aggregate-last):
  - Layer 1 "aggregate-first": A(xW1+b1) = (A x)W1 + (A 1)b1. Each core
    gathers pre-scaled x rows (dinv_s * x_s, bf16, 256B rows) directly from
    a DRAM parameter table in TRANSPOSE mode (features land on partitions,
    one gathered row per column), so layer 1 needs NO collective and no
    PE transpose: u[feat, dst] tiles come out of the segment reduce ready
    for the W1 matmul.
  - Scales factor as h1 = dinv_d * relu(w + gamma'_d b1) with
    w = (sum dinv_s x_s)W1; the per-column gamma' bias enters via a PE
    outer-product accumulated into the same PSUM tile, and both dinv_d
    factors are folded into the z scale (z = 64 dinv_d^2 (relu_part W2)).
  - Layer 2 "aggregate-last": out = dinv_d/64 * (sum_s z_s) + gamma_d b2.
    z rows (40 cols) are AllGathered TIGHT in fp8 (4MB -> ~115us vs 284us
    for the old f32 AllGather), in 4 row-chunks so the early chunks hide
    under the layer-1 gather tail, then expanded to a 256B-pitch DRAM
    table for the (non-transpose) fp8 gathers.
  - Same padded (tile,class) slot structure for both layers: identical
    descriptor counts, one idx SBUF buffer reused (idx2 uploaded during
    the AllGather window).
"""

import sys
import numpy as np

sys.path.insert(0, "/opt/trn_rl_repo")

N = 100000
E = 1600000
NFEAT, NHID, NCLASS = 128, 64, 40
NCORES = 8
CPN = 12544            # dst nodes per core (98 tiles of 128)
BLK = CPN + 1          # table block rows per core (+1 zero pad row)
NT = CPN // 128        # 98 tiles
NCLS = 4
CLS_ROWS = 2 * BLK     # 25090 table rows per class (= 2 core blocks)
TBL = NCORES * BLK     # 100360
PAD_LOCAL = CPN        # class-local index of the zero row
GB_SLOTS = 48          # max slots per (group, class) call
TG = 16                # max tiles per group
RING1 = 3              # L1 gather ring buffers
RING2 = 3              # L2 gather ring buffers
U_RING = 24            # u_sb ring depth (> TG + relu lag)
NCHUNK = 4             # AllGather row chunks
AG_MARGIN = 12
AG_R = [0, 4608, 8704, 11520, BLK]
F2 = 256               # z table row pitch (fp8 elems = bytes)


def _host_prep(edge_index):
    """Class assignment + per-core padded gather streams for both layers."""
    src0 = edge_index[0].astype(np.int64)
    dst0 = edge_index[1].astype(np.int64)
    loops = np.arange(N, dtype=np.int64)
    src = np.concatenate([src0, loops])          # self-loops in the stream
    dst = np.concatenate([dst0, loops])
    deg = np.bincount(src, minlength=N)
    dinv = (1.0 / np.sqrt(deg.astype(np.float64))).astype(np.float32)
    gsum = np.zeros(N, np.float64)
    np.add.at(gsum, dst, dinv[src].astype(np.float64))
    gprime = gsum.astype(np.float32)             # gamma' = sum_in dinv_s

    # ---- greedy class assignment of sources (balance each dst's in-nbrs) ----
    order_e = np.argsort(src, kind="stable")
    d_sorted = dst[order_e]
    sptr = np.searchsorted(src[order_e], np.arange(N + 1))
    cap = NCORES * CPN // NCLS                    # 25088 real nodes max per class
    cnt = np.zeros((N, NCLS), np.int32)
    cls = np.full(N, -1, np.int8)
    szs = np.zeros(NCLS, np.int64)
    outdeg = np.bincount(src, minlength=N)
    sorder = np.argsort(-outdeg, kind="stable")
    for s in sorder:
        dd = d_sorted[sptr[s]:sptr[s + 1]]
        sc = (4.0 ** cnt[dd, :]).sum(0)
        sc = sc + (szs >= cap) * 1e30
        c = int(sc.argmin())
        cls[s] = c
        szs[c] += 1
        cnt[dd, c] += 1

    # ---- refinement sweep: re-place each node with exact counts ----
    cnt = np.zeros((N, NCLS), np.int32)
    np.add.at(cnt, (dst, cls[src]), 1)
    for v in np.argsort(-outdeg, kind="stable"):
        dd = d_sorted[sptr[v]:sptr[v + 1]]
        c0 = cls[v]
        np.add.at(cnt, (dd, c0), -1)
        szs[c0] -= 1
        sc = (4.0 ** cnt[dd, :]).sum(0) + (szs >= cap) * 1e30
        c = int(sc.argmin())
        cls[v] = c
        szs[c] += 1
        np.add.at(cnt, (dd, c), 1)

    # ---- node -> (core, position): cluster similar in-profiles per tile ----
    blocks = []
    for c in range(NCLS):
        nodes_c = np.flatnonzero(cls == c)
        cc = cnt[nodes_c]
        order = np.lexsort((cc[:, 3], cc[:, 2], cc[:, 1], cc[:, 0],
                            cc.argmax(1), cc.max(1)))
        nodes_c = nodes_c[order]
        a = np.full(CPN, -1, np.int64)
        b = np.full(CPN, -1, np.int64)
        a[: (len(nodes_c) + 1) // 2] = nodes_c[0::2]
        b[: len(nodes_c) // 2] = nodes_c[1::2]
        blocks.append(a)
        blocks.append(b)

    row = np.full(N, -1, np.int64)
    for k in range(NCORES):
        blk = blocks[k]
        real = blk >= 0
        row[blk[real]] = k * BLK + np.flatnonzero(real)

    dcore = np.empty(N, np.int64)
    dpos = np.empty(N, np.int64)
    for k in range(NCORES):
        blk = blocks[k]
        real = blk >= 0
        dcore[blk[real]] = k
        dpos[blk[real]] = np.flatnonzero(real)
    ecore = dcore[dst]
    epos = dpos[dst]
    etile = epos // 128
    epart = epos % 128
    ecls = cls[src].astype(np.int64)
    esrow = row[src] - ecls * CLS_ROWS            # class-local table row
    assert esrow.min() >= 0 and esrow.max() < CLS_ROWS

    key = ((ecore * NT + etile) * NCLS + ecls) * 128 + epart
    eorder = np.argsort(key, kind="stable")
    key_s = key[eorder]
    esrow_s = esrow[eorder]
    counts = np.bincount(key_s, minlength=NCORES * NT * NCLS * 128)
    counts = counts.reshape(NCORES, NT, NCLS, 128)
    kmax = counts.max(axis=(0, 3))                # K per (tile, class)
    kmax = np.maximum(kmax, 1)

    # ---- call grouping: consecutive tiles, per-class slot sum <= GB_SLOTS ----
    groups = []
    cur = []
    for t in range(NT):
        trial = cur + [t]
        if cur and (len(trial) > TG or
                    max(kmax[trial, c].sum() for c in range(NCLS)) > GB_SLOTS):
            groups.append(cur)
            cur = [t]
        else:
            cur = trial
        if kmax[t].max() > GB_SLOTS:
            raise RuntimeError("single tile exceeds gather buffer")
    groups.append(cur)
    if len(groups[-1]) > 3:                       # short tail -> short drain
        groups.append(groups[-1][-2:])
        groups[-2] = groups[-2][:-2]

    # calls: (class, tiles, seg_offsets(slots), nslots)
    calls = []
    for g in groups:
        for c in range(NCLS):
            offs = np.concatenate([[0], np.cumsum(kmax[g, c])])
            calls.append((c, list(g), offs[:-1].tolist(), int(offs[-1])))
    total_slots = sum(nsl for (_, _, _, nsl) in calls)

    # ---- per-core index streams (both layers), vectorized ----
    flat_counts = counts.reshape(-1)
    starts = np.concatenate([[0], np.cumsum(flat_counts)])[:-1].reshape(
        NCORES, NT, NCLS, 128)
    epart_s = epart[eorder]

    streams1 = []   # L1 transpose-gather: column order (tile, dstpart, k)
    streams2 = []   # L2 gather: slot-major, partition fastest
    for k in range(NCORES):
        s1 = np.full(total_slots * 128, PAD_LOCAL, np.int16)
        s2 = np.full(total_slots * 128, PAD_LOCAL, np.int16)
        base = 0
        for (c, tiles, offs, nsl) in calls:
            for t, off in zip(tiles, offs):
                K = int(kmax[t, c])
                cnts = counts[k, t, c]            # [128]
                st = starts[k, t, c]
                tot = int(cnts.sum())
                if tot:
                    sl = slice(st[0], st[0] + tot)
                    parts = epart_s[sl]
                    jj = np.arange(tot) - np.repeat(st - st[0], cnts)
                    vals = esrow_s[sl].astype(np.int16)
                    # L2: slot-major within call, partition fastest
                    s2[(base + off + jj) * 128 + parts] = vals
                    # L1: columns (d * K + jj) at col base 128*(base+off)
                    s1[128 * (base + off) + parts * K + jj] = vals
            base += nsl
        streams1.append(np.tile(s1.reshape(-1, 16).T, (8, 1)))
        streams2.append(np.tile(s2.reshape(-1, 16).T, (8, 1)))

    pad_frac = total_slots * 128 / float(len(src) / NCORES) - 1.0
    meta = dict(blocks=blocks, row=row, dinv=dinv, gprime=gprime,
                calls=calls, groups=groups, total_slots=total_slots,
                kmax=kmax, pad_frac=pad_frac, dpos=dpos, dcore=dcore)
    return meta, streams1, streams2


def _build_program(calls, groups, kmax, stop_after="full"):
    import concourse.bacc as bacc
    import concourse.bass as bass
    from concourse import mybir
    from concourse.library_config import mlp
    from contextlib import ExitStack
    do_ag = stop_after in ("ag", "full")
    do_l2 = stop_after == "full"

    AF = mybir.ActivationFunctionType
    OP = mybir.AluOpType
    nc = bacc.Bacc("TRN2", target_bir_lowering=False, debug=False)

    NCALLS = len(calls)
    NG = len(groups)
    total_slots = sum(nsl for (_, _, _, nsl) in calls)
    ICOLS = total_slots * 8                       # int16 idx cols per partition
    # chunked AllGather row boundaries
    R = AG_R

    # idx chunk boundaries (in calls) for chunked idx1 upload: small first
    q_calls = [0, 2, 6, 12, 20, NCALLS // 2, 3 * NCALLS // 4, NCALLS]
    NQCH = len(q_calls) - 1
    call_off = np.concatenate([[0], np.cumsum([c[3] for c in calls])])
    q_off = [int(call_off[q]) * 8 for q in q_calls]   # idx col boundaries

    xtbl = nc.declare_dram_parameter("xtbl", [TBL, NFEAT], mybir.dt.bfloat16, isOutput=False)
    idx1p = nc.declare_dram_parameter("idx1", [128, ICOLS], mybir.dt.int16, isOutput=False)
    idx2p = nc.declare_dram_parameter("idx2", [128, ICOLS], mybir.dt.int16, isOutput=False)
    w1tp = nc.declare_dram_parameter("w1t", [128, NHID], mybir.dt.bfloat16, isOutput=False)
    b1cp = nc.declare_dram_parameter("b1c", [1, NHID], mybir.dt.bfloat16, isOutput=False)
    w2ap = nc.declare_dram_parameter("w2a", [NHID, NCLASS], mybir.dt.bfloat16, isOutput=False)
    gpbp = nc.declare_dram_parameter("gpb", [1, CPN], mybir.dt.bfloat16, isOutput=False)
    dvc2p = nc.declare_dram_parameter("dvc2", [128, NT], mybir.dt.float32, isOutput=False)
    dvc3p = nc.declare_dram_parameter("dvc3", [128, NT], mybir.dt.float32, isOutput=False)
    gb2p = nc.declare_dram_parameter("gb2", [128, NT * NCLASS], mybir.dt.bfloat16, isOutput=False)
    zrop = nc.declare_dram_parameter("zro", [1, NCLASS], mybir.dt.float8e4, isOutput=False)
    outp = nc.declare_dram_parameter("out", [CPN, NCLASS], mybir.dt.float32, isOutput=True)

    z_own = nc.dram_tensor("z_own", [BLK, NCLASS], mybir.dt.float8e4)
    z_full = nc.dram_tensor("z_full", [TBL, NCLASS], mybir.dt.float8e4, addr_space="Shared")
    ztbl = nc.dram_tensor("ztbl", [TBL, F2], mybir.dt.float8e4)

    with ExitStack() as stack:
        ec = stack.enter_context
        block = ec(nc.Block())
        idx_sb = ec(nc.sbuf_tensor("idx_sb", [128, ICOLS], mybir.dt.int16))
        gbuf1 = ec(nc.sbuf_tensor("gbuf1", [128, RING1, 1, GB_SLOTS * 128], mybir.dt.bfloat16))
        gbuf2 = ec(nc.sbuf_tensor("gbuf2", [128, RING2, GB_SLOTS, F2], mybir.dt.float8e4))
        up4 = ec(nc.sbuf_tensor("up4", [128, TG, NCLS, 128], mybir.dt.bfloat16))
        o4 = ec(nc.sbuf_tensor("o4", [128, TG, NCLS, NCLASS], mybir.dt.float32))
        u_sb = ec(nc.sbuf_tensor("u_sb", [128, U_RING, 128], mybir.dt.bfloat16))
        tA = ec(nc.sbuf_tensor("tA", [128, 128], mybir.dt.bfloat16))
        tB = ec(nc.sbuf_tensor("tB", [128, 128], mybir.dt.bfloat16))
        oA = ec(nc.sbuf_tensor("oA", [128, NCLASS], mybir.dt.float32))
        oB = ec(nc.sbuf_tensor("oB", [128, NCLASS], mybir.dt.float32))
        vT = ec(nc.sbuf_tensor("vT", [NHID, 2, 128], mybir.dt.bfloat16))
        zsb = ec(nc.sbuf_tensor("zsb", [128, 4, NCLASS], mybir.dt.float8e4))
        ob = ec(nc.sbuf_tensor("ob", [128, NT, NCLASS], mybir.dt.float32))
        w1t_sb = ec(nc.sbuf_tensor("w1t_sb", [128, NHID], mybir.dt.bfloat16))
        b1c_sb = ec(nc.sbuf_tensor("b1c_sb", [1, NHID], mybir.dt.bfloat16))
        w2a_sb = ec(nc.sbuf_tensor("w2a_sb", [NHID, NCLASS], mybir.dt.bfloat16))
        gpb_sb = ec(nc.sbuf_tensor("gpb_sb", [1, CPN], mybir.dt.bfloat16))
        dvc2_sb = ec(nc.sbuf_tensor("dvc2_sb", [128, NT], mybir.dt.float32))
        dvc3_sb = ec(nc.sbuf_tensor("dvc3_sb", [128, NT], mybir.dt.float32))
        gb2_sb = ec(nc.sbuf_tensor("gb2_sb", [128, NT, NCLASS], mybir.dt.bfloat16))
        zro_sb = ec(nc.sbuf_tensor("zro_sb", [1, NCLASS], mybir.dt.float8e4))
        tmp40 = ec(nc.sbuf_tensor("tmp40", [128, NCLASS], mybir.dt.float32))
        lse = ec(nc.sbuf_tensor("lse", [128, NT], mybir.dt.float32))
        lnl = ec(nc.sbuf_tensor("lnl", [128, NT], mybir.dt.float32))
        ph = ec(nc.psum_tensor("ph", [NHID, 2, 512], mybir.dt.float32))
        pz = ec(nc.psum_tensor("pz", [128, 2, 512], mybir.dt.float32))

        sems = {}
        for n in ["s_c", "s_i2", "s_red1", "s_red2", "s_u", "s_h", "s_v1",
                  "s_mm2", "s_zq", "s_zst", "s_cc", "s_exp", "s_ob", "s_ea",
                  "s_ln", "s_sm", "s_out"]:
            sems[n] = ec(nc.semaphore(n))
        (s_c, s_i2, s_red1, s_red2, s_u, s_h, s_v1, s_mm2, s_zq, s_zst,
         s_cc, s_exp, s_ob, s_ea, s_ln, s_sm, s_out) = (
            sems[n] for n in ["s_c", "s_i2", "s_red1", "s_red2", "s_u", "s_h",
                              "s_v1", "s_mm2", "s_zq", "s_zst", "s_cc",
                              "s_exp", "s_ob", "s_ea", "s_ln", "s_sm", "s_out"])
        s_i1 = [ec(nc.semaphore(f"s_i1_{q}")) for q in range(NQCH)]
        s_gb1 = [ec(nc.semaphore(f"s_gb1_{b}")) for b in range(RING1)]
        s_zs = [ec(nc.semaphore(f"s_zs_{b}")) for b in range(4)]
        s_zp = ec(nc.semaphore("s_zp"))
        s_gb2 = [ec(nc.semaphore(f"s_gb2_{b}")) for b in range(RING2)]

        # global tile order: tiles listed group by group (= 0..NT-1)
        tile_group = {}
        for gi, g in enumerate(groups):
            for t in g:
                tile_group[t] = gi

        # ---------------- sync engine: uploads + stores ----------------
        @block.sync
        def _(se: bass.BassEngine):
            for q in range(NQCH):
                se.dma_start(idx_sb[:, q_off[q]:q_off[q + 1]],
                             idx1p[:, q_off[q]:q_off[q + 1]]).then_inc(s_i1[q], 16)
            se.dma_start(w1t_sb[:], w1tp[:]).then_inc(s_c, 16)
            se.dma_start(b1c_sb[:], b1cp[:]).then_inc(s_c, 16)
            se.dma_start(w2a_sb[:], w2ap[:]).then_inc(s_c, 16)
            se.dma_start(gpb_sb[:], gpbp[:]).then_inc(s_c, 16)
            se.dma_start(dvc2_sb[:], dvc2p[:]).then_inc(s_c, 16)
            se.dma_start(dvc3_sb[:], dvc3p[:]).then_inc(s_c, 16)
            se.dma_start(gb2_sb[:], gb2p[:].rearrange("p (t f) -> p t f", f=NCLASS)).then_inc(s_c, 16)
            se.dma_start(zro_sb[:], zrop[:]).then_inc(s_c, 16)
            # z pad row (zro upload must land first)
            se.wait_ge(s_c, 128)
            se.dma_start(z_own[CPN:CPN + 1, :], zro_sb[:]).then_inc(s_zp, 16)
            # z stores, one per tile
            for t in range(NT):
                se.wait_ge(s_zq, t + 1)
                se.dma_start(z_own[128 * t:128 * (t + 1), :],
                             zsb[:, t % 4, :]).then_inc(s_zs[t % 4], 16)
            if do_ag:
                # idx2 upload (gathers all consumed by then)
                se.wait_ge(s_red1, NCALLS)
                se.dma_start(idx_sb[:], idx2p[:]).then_inc(s_i2, 16)
            if do_l2:
                # output stores, one per group
                done = 0
                for gi, g in enumerate(groups):
                    done += len(g)
                    se.wait_ge(s_sm, done)
                    t0, t1 = g[0], g[-1] + 1
                    dst_ap = outp[t0 * 128:t1 * 128, :].rearrange("(k p) f -> p k f", p=128)
                    se.dma_start(dst_ap, ob[:, t0:t1, :]).then_inc(s_out, 16)
                se.wait_ge(s_out, 16 * NG)
            elif do_ag:
                se.wait_ge(s_exp, 16 * 8 * NCLS)
            else:
                se.wait_ge(s_zp, 16)
                for sl in range(4):
                    se.wait_ge(s_zs[sl], 16 * ((NT - sl + 3) // 4))

        # ---------------- gpsimd: gathers + collectives ----------------
        @block.gpsimd
        def _(g: bass.BassGpSimd):
            g.load_library(mlp)
            # AG chunk issue positions (call index) and boundary tiles
            bounds = [R[ch + 1] // 128 for ch in range(NCHUNK)]
            # earliest call index at which tile `b-1`'s group is fully issued
            def _call_of(b):
                for gi, g in enumerate(groups):
                    if b - 1 in g:
                        return (gi + 1) * NCLS
                return NCALLS
            ag_pos = [min(_call_of(bounds[ch]) + AG_MARGIN, NCALLS)
                      for ch in range(NCHUNK)]

            def _ag(ch):
                g.wait_ge(s_zp, 16)
                M = min(NT, bounds[ch] + 4)
                for sl in range(4):
                    need = (M - sl + 3) // 4
                    if need > 0:
                        g.wait_ge(s_zs[sl], 16 * need)
                g.collective_compute(
                    "AllGather", mybir.AluOpType.bypass,
                    replica_groups=[list(range(NCORES))],
                    ins=[z_own[R[ch]:R[ch + 1], :].opt()],
                    outs=[z_full[8 * R[ch]:8 * R[ch + 1], :].opt()],
                ).then_inc(s_cc)

            # layer 1 gathers (transpose mode)
            off = 0
            for j, (c, tiles, offs, nsl) in enumerate(calls):
                if do_ag:
                    for ch in range(NCHUNK):
                        if ag_pos[ch] == j:
                            _ag(ch)
                q = next(qq for qq in range(NQCH) if q_calls[qq + 1] > j)
                g.wait_ge(s_i1[q], 16)
                if j >= RING1:
                    g.wait_ge(s_red1, j - RING1 + 1)
                cols = nsl * 128
                g.dma_gather(
                    gbuf1[:, j % RING1, :, 0:cols],
                    xtbl[c * CLS_ROWS:(c + 1) * CLS_ROWS, :],
                    idx_sb[:, off * 8:(off + nsl) * 8],
                    cols, cols, NFEAT,
                    transpose=True,
                    single_packet=False,
                ).then_inc(s_gb1[j % RING1], 16)
                off += nsl
            if do_ag:
                for ch in range(NCHUNK):
                    if ag_pos[ch] >= NCALLS:
                        _ag(ch)
            # expansion: chunk-major so pieces run as their chunk lands
            for ch in (range(NCHUNK) if do_ag else []):
                Lc = R[ch + 1] - R[ch]
                g.wait_ge(s_cc, ch + 1)
                for k in range(NCORES):
                    g.dma_start(
                        ztbl[k * BLK + R[ch]:k * BLK + R[ch] + Lc, 0:NCLASS],
                        z_full[8 * R[ch] + k * Lc:8 * R[ch] + (k + 1) * Lc, :],
                    ).then_inc(s_exp, 16)
            # layer 2 gathers
            if do_l2:
                g.wait_ge(s_i2, 16)
            off = 0
            for j, (c, tiles, offs, nsl) in enumerate(calls if do_l2 else []):
                g.wait_ge(s_exp, 16 * 8 * NCHUNK)
                if j >= RING2:
                    g.wait_ge(s_red2, j - RING2 + 1)
                nidx = nsl * 128
                g.dma_gather(
                    gbuf2[:, j % RING2, 0:nsl, :],
                    ztbl[c * CLS_ROWS:(c + 1) * CLS_ROWS, :],
                    idx_sb[:, off * 8:(off + nsl) * 8],
                    nidx, nidx, F2,
                    single_packet=False,
                ).then_inc(s_gb2[j % RING2], 16)
                off += nsl

        # ---------------- tensor engine ----------------
        @block.tensor
        def _(te):
            te.wait_ge(s_c, 128)
            for t in range(NT):
                te.wait_ge(s_u, t + 1)
                if t >= 2:
                    te.wait_ge(s_v1, t - 1)      # ph ring slot free
                te.matmul(ph[:, t % 2, 0:128], w1t_sb[:, :], u_sb[:, t % U_RING, :],
                          start=True, stop=False)
                te.matmul(ph[:, t % 2, 0:128], b1c_sb[0:1, :],
                          gpb_sb[0:1, 128 * t:128 * (t + 1)],
                          start=False, stop=True).then_inc(s_h)
                if t >= 1:
                    te.wait_ge(s_v1, t)
                    if t >= 3:
                        te.wait_ge(s_zq, t - 2)  # pz ring slot free
                    te.matmul(pz[:, (t - 1) % 2, 0:NCLASS],
                              vT[:, (t - 1) % 2, :], w2a_sb[:, :],
                              start=True, stop=True).then_inc(s_mm2)
            te.wait_ge(s_v1, NT)
            te.wait_ge(s_zq, NT - 2)
            te.matmul(pz[:, (NT - 1) % 2, 0:NCLASS],
                      vT[:, (NT - 1) % 2, :], w2a_sb[:, :],
                      start=True, stop=True).then_inc(s_mm2)

        # ---------------- vector engine ----------------
        @block.vector
        def _(v: bass.BassVectorEngine):
            v.wait_ge(s_c, 128)

            # ---- layer 1 ----
            for gi, g in enumerate(groups):
                for c in range(NCLS):
                    j = gi * NCLS + c
                    (_, tiles, offs, nsl) = calls[j]
                    v.wait_ge(s_gb1[j % RING1], 16 * (j // RING1 + 1))
                    with nc.allow_low_precision(reason="bf16 partial aggregates"):
                        for ti, t in enumerate(tiles):
                            K = int(kmax[t, c])
                            seg = gbuf1[:, j % RING1, 0,
                                        128 * offs[ti]:128 * (offs[ti] + K)]
                            seg = seg.rearrange("p (d k) -> p d k", k=K)
                            v.tensor_reduce(up4[:, ti, c, :], seg,
                                            axis=mybir.AxisListType.X, op=OP.add)
                    v.nop().then_inc(s_red1, 1)
                for ti, t in enumerate(g):
                    if t >= U_RING:
                        v.wait_ge(s_h, t - U_RING + 1)   # u_sb ring slot free
                    v.tensor_add(tA[:, :], up4[:, ti, 0, :], up4[:, ti, 1, :])
                    v.tensor_add(tB[:, :], up4[:, ti, 2, :], up4[:, ti, 3, :])
                    v.tensor_add(u_sb[:, t % U_RING, :], tA[:, :], tB[:, :]).then_inc(s_u)
            # ---- layer 2 ----
            def _final(gi):
                for t in groups[gi]:
                    v.wait_ge(s_ln, gi + 1)
                    v.tensor_scalar(out=ob[:, t, :], in0=ob[:, t, :],
                                    scalar1=lnl[:, t:t + 1], scalar2=None,
                                    op0=OP.subtract).then_inc(s_sm)

            for gi, g in enumerate(groups if do_l2 else []):
                for c in range(NCLS):
                    j = gi * NCLS + c
                    (_, tiles, offs, nsl) = calls[j]
                    v.wait_ge(s_gb2[j % RING2], 16 * (j // RING2 + 1))
                    for ti, t in enumerate(tiles):
                        K = int(kmax[t, c])
                        seg = gbuf2[:, j % RING2, offs[ti]:offs[ti] + K, 0:NCLASS]
                        seg = seg.rearrange("p k f -> p f k")
                        v.tensor_reduce(o4[:, ti, c, :], seg,
                                        axis=mybir.AxisListType.X, op=OP.add)
                    v.nop().then_inc(s_red2, 1)
                for ti, t in enumerate(g):
                    v.tensor_add(oA[:, :], o4[:, ti, 0, :], o4[:, ti, 1, :])
                    v.tensor_add(oB[:, :], o4[:, ti, 2, :], o4[:, ti, 3, :])
                    v.tensor_add(oA[:, :], oA[:, :], oB[:, :])
                    v.scalar_tensor_tensor(
                        out=ob[:, t, :], in0=oA[:, :],
                        scalar=dvc3_sb[:, t:t + 1], in1=gb2_sb[:, t, :],
                        op0=OP.mult, op1=OP.add).then_inc(s_ob)
                if gi >= 1:
                    _final(gi - 1)
            if do_l2:
                _final(NG - 1)

        # ---------------- scalar engine (Act) ----------------
        @block.scalar
        def _(sc):
            sc.wait_ge(s_c, 128)
            # layer 1: relu + z quantize, software-pipelined per tile
            def _zq(t):
                sc.wait_ge(s_mm2, t + 1)
                if t >= 4:
                    sc.wait_ge(s_zs[t % 4], 16 * ((t - 4) // 4 + 1))  # slot free
                sc.activation(zsb[:, t % 4, :], pz[:, t % 2, 0:NCLASS],
                              AF.Copy, scale=dvc2_sb[:, t:t + 1]).then_inc(s_zq)

            for t in range(NT):
                sc.wait_ge(s_h, t + 1)
                if t >= 2:
                    sc.wait_ge(s_mm2, t - 1)      # vT ring slot free
                sc.activation(vT[:, t % 2, :], ph[:, t % 2, 0:128],
                              AF.Relu).then_inc(s_v1)
                if t >= 1:
                    _zq(t - 1)
            _zq(NT - 1)
            # layer 2: exp accumulate + per-group Ln
            nea = 0
            for gi, g in enumerate(groups if do_l2 else []):
                for t in g[:-1]:
                    sc.wait_ge(s_ob, t + 1)
                    sc.activation(tmp40[:], ob[:, t, :], AF.Exp,
                                  accum_out=lse[:, t:t + 1])
                t = g[-1]
                sc.wait_ge(s_ob, t + 1)
                sc.activation(tmp40[:], ob[:, t, :], AF.Exp,
                              accum_out=lse[:, t:t + 1]).then_inc(s_ea)
                nea += 1
                sc.wait_ge(s_ea, nea)
                t0, t1 = g[0], g[-1] + 1
                sc.activation(lnl[:, t0:t1], lse[:, t0:t1], AF.Ln).then_inc(s_ln)

    nc.compile()
    return nc


_LAST_NC = None


def kernel(x, W1, b1, W2, b2, edge_index):
    global _LAST_NC
    from concourse.bass_utils import run_bass_kernel_spmd
    import ml_dtypes

    x = np.asarray(x)
    W1 = np.asarray(W1); b1 = np.asarray(b1)
    W2 = np.asarray(W2); b2 = np.asarray(b2)
    edge_index = np.asarray(edge_index)

    meta, streams1, streams2 = _host_prep(edge_index)
    calls = meta["calls"]
    groups = meta["groups"]
    kmax = meta["kmax"]
    nc = _build_program(calls, groups, kmax)
    _LAST_NC = nc

    dinv = meta["dinv"]
    gprime = meta["gprime"]
    blocks = meta["blocks"]

    # x table: row k*BLK+pos = dinv[n] * x[n] (bf16); pad rows zero
    xtbl = np.zeros((TBL, NFEAT), ml_dtypes.bfloat16)
    for k in range(NCORES):
        blk = blocks[k]
        real = np.flatnonzero(blk >= 0)
        nn = blk[real]
        xtbl[k * BLK + real] = (dinv[nn][:, None] * x[nn]).astype(ml_dtypes.bfloat16)

    w1t_np = W1.T.astype(ml_dtypes.bfloat16)                 # [128, 64]
    b1c_np = b1.reshape(1, NHID).astype(ml_dtypes.bfloat16)
    w2a_np = W2.T.astype(ml_dtypes.bfloat16)                 # [64, 40]
    zro_np = np.zeros((1, NCLASS), ml_dtypes.float8_e4m3fn)

    in_maps = []
    for k in range(NCORES):
        blk = blocks[k]
        real = blk >= 0
        idxs = np.flatnonzero(real)
        nn = blk[idxs]
        gpb_np = np.zeros(CPN, np.float32)
        gpb_np[idxs] = gprime[nn]
        dvc2_np = np.zeros(CPN, np.float32)
        dvc2_np[idxs] = 64.0 * dinv[nn] * dinv[nn]
        dvc3_np = np.zeros(CPN, np.float32)
        dvc3_np[idxs] = dinv[nn] / 64.0
        gb2_np = np.zeros((CPN, NCLASS), np.float32)
        gb2_np[idxs] = (dinv[nn] * gprime[nn])[:, None] * b2[None, :]
        in_maps.append({
            "xtbl": xtbl,
            "idx1": streams1[k], "idx2": streams2[k],
            "w1t": w1t_np, "b1c": b1c_np, "w2a": w2a_np,
            "gpb": gpb_np.reshape(1, CPN).astype(ml_dtypes.bfloat16),
            "dvc2": dvc2_np.reshape(NT, 128).T.copy(),
            "dvc3": dvc3_np.reshape(NT, 128).T.copy(),
            "gb2": gb2_np.reshape(NT, 128, NCLASS).transpose(1, 0, 2)
                        .reshape(128, NT * NCLASS).astype(ml_dtypes.bfloat16),
            "zro": zro_np,
        })

    res = run_bass_kernel_spmd(nc, in_maps, list(range(NCORES)))

    out = np.empty((N, NCLASS), np.float32)
    for k in range(NCORES):
        blk = blocks[k]
        real = blk >= 0
        out[blk[real]] = res.results[k]["out"][np.flatnonzero(real)]
    return out


# revision 21
# speedup vs baseline: 1.0025x; 1.0025x over previous
"""2-layer GCN (gnn_message_passing) on 8 Trainium2 NeuronCores.

Strategy (v3 - aggregate-first/aggregate-last):
  - Layer 1 "aggregate-first": A(xW1+b1) = (A x)W1 + (A 1)b1. Each core
    gathers pre-scaled x rows (dinv_s * x_s, bf16, 256B rows) directly from
    a DRAM parameter table in TRANSPOSE mode (features land on partitions,
    one gathered row per column), so layer 1 needs NO collective and no
    PE transpose: u[feat, dst] tiles come out of the segment reduce ready
    for the W1 matmul.
  - Scales factor as h1 = dinv_d * relu(w + gamma'_d b1) with
    w = (sum dinv_s x_s)W1; the per-column gamma' bias enters via a PE
    outer-product accumulated into the same PSUM tile, and both dinv_d
    factors are folded into the z scale (z = 64 dinv_d^2 (relu_part W2)).
  - Layer 2 "aggregate-last": out = dinv_d/64 * (sum_s z_s) + gamma_d b2.
    z rows (40 cols) are AllGathered TIGHT in fp8 (4MB -> ~115us vs 284us
    for the old f32 AllGather), in 4 row-chunks so the early chunks hide
    under the layer-1 gather tail, then expanded to a 256B-pitch DRAM
    table for the (non-transpose) fp8 gathers.
  - Same padded (tile,class) slot structure for both layers: identical
    descriptor counts, one idx SBUF buffer reused (idx2 uploaded during
    the AllGather window).
"""

import sys
import numpy as np

sys.path.insert(0, "/opt/trn_rl_repo")

N = 100000
E = 1600000
NFEAT, NHID, NCLASS = 128, 64, 40
NCORES = 8
CPN = 12544            # dst nodes per core (98 tiles of 128)
BLK = CPN + 1          # table block rows per core (+1 zero pad row)
NT = CPN // 128        # 98 tiles
NCLS = 4
CLS_ROWS = 2 * BLK     # 25090 table rows per class (= 2 core blocks)
TBL = NCORES * BLK     # 100360
PAD_LOCAL = CPN        # class-local index of the zero row
GB_SLOTS = 48          # max slots per (group, class) call
TG = 16                # max tiles per group
RING1 = 3              # L1 gather ring buffers
RING2 = 3              # L2 gather ring buffers
U_RING = 24            # u_sb ring depth (> TG + relu lag)
NCHUNK = 4             # AllGather row chunks
AG_MARGIN = 12
AG_R = [0, 4864, 8960, 11648, BLK]
F2 = 256               # z table row pitch (fp8 elems = bytes)


def _host_prep(edge_index):
    """Class assignment + per-core padded gather streams for both layers."""
    src0 = edge_index[0].astype(np.int64)
    dst0 = edge_index[1].astype(np.int64)
    loops = np.arange(N, dtype=np.int64)
    src = np.concatenate([src0, loops])          # self-loops in the stream
    dst = np.concatenate([dst0, loops])
    deg = np.bincount(src, minlength=N)
    dinv = (1.0 / np.sqrt(deg.astype(np.float64))).astype(np.float32)
    gsum = np.zeros(N, np.float64)
    np.add.at(gsum, dst, dinv[src].astype(np.float64))
    gprime = gsum.astype(np.float32)             # gamma' = sum_in dinv_s

    # ---- greedy class assignment of sources (balance each dst's in-nbrs) ----
    order_e = np.argsort(src, kind="stable")
    d_sorted = dst[order_e]
    sptr = np.searchsorted(src[order_e], np.arange(N + 1))
    cap = NCORES * CPN // NCLS                    # 25088 real nodes max per class
    cnt = np.zeros((N, NCLS), np.int32)
    cls = np.full(N, -1, np.int8)
    szs = np.zeros(NCLS, np.int64)
    outdeg = np.bincount(src, minlength=N)
    sorder = np.argsort(-outdeg, kind="stable")
    for s in sorder:
        dd = d_sorted[sptr[s]:sptr[s + 1]]
        sc = (4.0 ** cnt[dd, :]).sum(0)
        sc = sc + (szs >= cap) * 1e30
        c = int(sc.argmin())
        cls[s] = c
        szs[c] += 1
        cnt[dd, c] += 1

    # ---- refinement sweep: re-place each node with exact counts ----
    cnt = np.zeros((N, NCLS), np.int32)
    np.add.at(cnt, (dst, cls[src]), 1)
    for v in np.argsort(-outdeg, kind="stable"):
        dd = d_sorted[sptr[v]:sptr[v + 1]]
        c0 = cls[v]
        np.add.at(cnt, (dd, c0), -1)
        szs[c0] -= 1
        sc = (4.0 ** cnt[dd, :]).sum(0) + (szs >= cap) * 1e30
        c = int(sc.argmin())
        cls[v] = c
        szs[c] += 1
        np.add.at(cnt, (dd, c), 1)

    # ---- node -> (core, position): cluster similar in-profiles per tile ----
    blocks = []
    for c in range(NCLS):
        nodes_c = np.flatnonzero(cls == c)
        cc = cnt[nodes_c]
        order = np.lexsort((cc[:, 3], cc[:, 2], cc[:, 1], cc[:, 0],
                            cc.argmax(1), cc.max(1)))
        nodes_c = nodes_c[order]
        a = np.full(CPN, -1, np.int64)
        b = np.full(CPN, -1, np.int64)
        a[: (len(nodes_c) + 1) // 2] = nodes_c[0::2]
        b[: len(nodes_c) // 2] = nodes_c[1::2]
        blocks.append(a)
        blocks.append(b)

    row = np.full(N, -1, np.int64)
    for k in range(NCORES):
        blk = blocks[k]
        real = blk >= 0
        row[blk[real]] = k * BLK + np.flatnonzero(real)

    dcore = np.empty(N, np.int64)
    dpos = np.empty(N, np.int64)
    for k in range(NCORES):
        blk = blocks[k]
        real = blk >= 0
        dcore[blk[real]] = k
        dpos[blk[real]] = np.flatnonzero(real)
    ecore = dcore[dst]
    epos = dpos[dst]
    etile = epos // 128
    epart = epos % 128
    ecls = cls[src].astype(np.int64)
    esrow = row[src] - ecls * CLS_ROWS            # class-local table row
    assert esrow.min() >= 0 and esrow.max() < CLS_ROWS

    key = ((ecore * NT + etile) * NCLS + ecls) * 128 + epart
    eorder = np.argsort(key, kind="stable")
    key_s = key[eorder]
    esrow_s = esrow[eorder]
    counts = np.bincount(key_s, minlength=NCORES * NT * NCLS * 128)
    counts = counts.reshape(NCORES, NT, NCLS, 128)
    kmax = counts.max(axis=(0, 3))                # K per (tile, class)
    kmax = np.maximum(kmax, 1)

    # ---- call grouping: consecutive tiles, per-class slot sum <= GB_SLOTS ----
    groups = []
    cur = []
    for t in range(NT):
        trial = cur + [t]
        if cur and (len(trial) > TG or
                    max(kmax[trial, c].sum() for c in range(NCLS)) > GB_SLOTS):
            groups.append(cur)
            cur = [t]
        else:
            cur = trial
        if kmax[t].max() > GB_SLOTS:
            raise RuntimeError("single tile exceeds gather buffer")
    groups.append(cur)
    if len(groups[-1]) > 3:                       # short tail -> short drain
        groups.append(groups[-1][-2:])
        groups[-2] = groups[-2][:-2]

    # calls: (class, tiles, seg_offsets(slots), nslots)
    calls = []
    for g in groups:
        for c in range(NCLS):
            offs = np.concatenate([[0], np.cumsum(kmax[g, c])])
            calls.append((c, list(g), offs[:-1].tolist(), int(offs[-1])))
    total_slots = sum(nsl for (_, _, _, nsl) in calls)

    # ---- per-core index streams (both layers), vectorized ----
    flat_counts = counts.reshape(-1)
    starts = np.concatenate([[0], np.cumsum(flat_counts)])[:-1].reshape(
        NCORES, NT, NCLS, 128)
    epart_s = epart[eorder]

    streams1 = []   # L1 transpose-gather: column order (tile, dstpart, k)
    streams2 = []   # L2 gather: slot-major, partition fastest
    for k in range(NCORES):
        s1 = np.full(total_slots * 128, PAD_LOCAL, np.int16)
        s2 = np.full(total_slots * 128, PAD_LOCAL, np.int16)
        base = 0
        for (c, tiles, offs, nsl) in calls:
            for t, off in zip(tiles, offs):
                K = int(kmax[t, c])
                cnts = counts[k, t, c]            # [128]
                st = starts[k, t, c]
                tot = int(cnts.sum())
                if tot:
                    sl = slice(st[0], st[0] + tot)
                    parts = epart_s[sl]
                    jj = np.arange(tot) - np.repeat(st - st[0], cnts)
                    vals = esrow_s[sl].astype(np.int16)
                    # L2: slot-major within call, partition fastest
                    s2[(base + off + jj) * 128 + parts] = vals
                    # L1: columns (d * K + jj) at col base 128*(base+off)
                    s1[128 * (base + off) + parts * K + jj] = vals
            base += nsl
        streams1.append(np.tile(s1.reshape(-1, 16).T, (8, 1)))
        streams2.append(np.tile(s2.reshape(-1, 16).T, (8, 1)))

    pad_frac = total_slots * 128 / float(len(src) / NCORES) - 1.0
    meta = dict(blocks=blocks, row=row, dinv=dinv, gprime=gprime,
                calls=calls, groups=groups, total_slots=total_slots,
                kmax=kmax, pad_frac=pad_frac, dpos=dpos, dcore=dcore)
    return meta, streams1, streams2


def _build_program(calls, groups, kmax, stop_after="full"):
    import concourse.bacc as bacc
    import concourse.bass as bass
    from concourse import mybir
    from concourse.library_config import mlp
    from contextlib import ExitStack
    do_ag = stop_after in ("ag", "full")
    do_l2 = stop_after == "full"

    AF = mybir.ActivationFunctionType
    OP = mybir.AluOpType
    nc = bacc.Bacc("TRN2", target_bir_lowering=False, debug=False)

    NCALLS = len(calls)
    NG = len(groups)
    total_slots = sum(nsl for (_, _, _, nsl) in calls)
    ICOLS = total_slots * 8                       # int16 idx cols per partition
    # chunked AllGather row boundaries
    R = AG_R

    # idx chunk boundaries (in calls) for chunked idx1 upload: small first
    q_calls = [0, 2, 6, 12, 20, NCALLS // 2, 3 * NCALLS // 4, NCALLS]
    NQCH = len(q_calls) - 1
    call_off = np.concatenate([[0], np.cumsum([c[3] for c in calls])])
    q_off = [int(call_off[q]) * 8 for q in q_calls]   # idx col boundaries

    xtbl = nc.declare_dram_parameter("xtbl", [TBL, NFEAT], mybir.dt.bfloat16, isOutput=False)
    idx1p = nc.declare_dram_parameter("idx1", [128, ICOLS], mybir.dt.int16, isOutput=False)
    idx2p = nc.declare_dram_parameter("idx2", [128, ICOLS], mybir.dt.int16, isOutput=False)
    w1tp = nc.declare_dram_parameter("w1t", [128, NHID], mybir.dt.bfloat16, isOutput=False)
    b1cp = nc.declare_dram_parameter("b1c", [1, NHID], mybir.dt.bfloat16, isOutput=False)
    w2ap = nc.declare_dram_parameter("w2a", [NHID, NCLASS], mybir.dt.bfloat16, isOutput=False)
    gpbp = nc.declare_dram_parameter("gpb", [1, CPN], mybir.dt.bfloat16, isOutput=False)
    dvc2p = nc.declare_dram_parameter("dvc2", [128, NT], mybir.dt.float32, isOutput=False)
    dvc3p = nc.declare_dram_parameter("dvc3", [128, NT], mybir.dt.float32, isOutput=False)
    gb2p = nc.declare_dram_parameter("gb2", [128, NT * NCLASS], mybir.dt.bfloat16, isOutput=False)
    zrop = nc.declare_dram_parameter("zro", [1, NCLASS], mybir.dt.float8e4, isOutput=False)
    outp = nc.declare_dram_parameter("out", [CPN, NCLASS], mybir.dt.float32, isOutput=True)

    z_own = nc.dram_tensor("z_own", [BLK, NCLASS], mybir.dt.float8e4)
    z_full = nc.dram_tensor("z_full", [TBL, NCLASS], mybir.dt.float8e4, addr_space="Shared")
    ztbl = nc.dram_tensor("ztbl", [TBL, F2], mybir.dt.float8e4)

    with ExitStack() as stack:
        ec = stack.enter_context
        block = ec(nc.Block())
        idx_sb = ec(nc.sbuf_tensor("idx_sb", [128, ICOLS], mybir.dt.int16))
        gbuf1 = ec(nc.sbuf_tensor("gbuf1", [128, RING1, 1, GB_SLOTS * 128], mybir.dt.bfloat16))
        gbuf2 = ec(nc.sbuf_tensor("gbuf2", [128, RING2, GB_SLOTS, F2], mybir.dt.float8e4))
        up4 = ec(nc.sbuf_tensor("up4", [128, TG, NCLS, 128], mybir.dt.bfloat16))
        o4 = ec(nc.sbuf_tensor("o4", [128, TG, NCLS, NCLASS], mybir.dt.float32))
        u_sb = ec(nc.sbuf_tensor("u_sb", [128, U_RING, 128], mybir.dt.bfloat16))
        tA = ec(nc.sbuf_tensor("tA", [128, 128], mybir.dt.bfloat16))
        tB = ec(nc.sbuf_tensor("tB", [128, 128], mybir.dt.bfloat16))
        oA = ec(nc.sbuf_tensor("oA", [128, NCLASS], mybir.dt.float32))
        oB = ec(nc.sbuf_tensor("oB", [128, NCLASS], mybir.dt.float32))
        vT = ec(nc.sbuf_tensor("vT", [NHID, 2, 128], mybir.dt.bfloat16))
        zsb = ec(nc.sbuf_tensor("zsb", [128, 4, NCLASS], mybir.dt.float8e4))
        ob = ec(nc.sbuf_tensor("ob", [128, NT, NCLASS], mybir.dt.float32))
        w1t_sb = ec(nc.sbuf_tensor("w1t_sb", [128, NHID], mybir.dt.bfloat16))
        b1c_sb = ec(nc.sbuf_tensor("b1c_sb", [1, NHID], mybir.dt.bfloat16))
        w2a_sb = ec(nc.sbuf_tensor("w2a_sb", [NHID, NCLASS], mybir.dt.bfloat16))
        gpb_sb = ec(nc.sbuf_tensor("gpb_sb", [1, CPN], mybir.dt.bfloat16))
        dvc2_sb = ec(nc.sbuf_tensor("dvc2_sb", [128, NT], mybir.dt.float32))
        dvc3_sb = ec(nc.sbuf_tensor("dvc3_sb", [128, NT], mybir.dt.float32))
        gb2_sb = ec(nc.sbuf_tensor("gb2_sb", [128, NT, NCLASS], mybir.dt.bfloat16))
        zro_sb = ec(nc.sbuf_tensor("zro_sb", [1, NCLASS], mybir.dt.float8e4))
        tmp40 = ec(nc.sbuf_tensor("tmp40", [128, NCLASS], mybir.dt.float32))
        lse = ec(nc.sbuf_tensor("lse", [128, NT], mybir.dt.float32))
        lnl = ec(nc.sbuf_tensor("lnl", [128, NT], mybir.dt.float32))
        ph = ec(nc.psum_tensor("ph", [NHID, 2, 512], mybir.dt.float32))
        pz = ec(nc.psum_tensor("pz", [128, 2, 512], mybir.dt.float32))

        sems = {}
        for n in ["s_c", "s_i2", "s_red1", "s_red2", "s_u", "s_h", "s_v1",
                  "s_mm2", "s_zq", "s_zst", "s_cc", "s_exp", "s_ob", "s_ea",
                  "s_ln", "s_sm", "s_out"]:
            sems[n] = ec(nc.semaphore(n))
        (s_c, s_i2, s_red1, s_red2, s_u, s_h, s_v1, s_mm2, s_zq, s_zst,
         s_cc, s_exp, s_ob, s_ea, s_ln, s_sm, s_out) = (
            sems[n] for n in ["s_c", "s_i2", "s_red1", "s_red2", "s_u", "s_h",
                              "s_v1", "s_mm2", "s_zq", "s_zst", "s_cc",
                              "s_exp", "s_ob", "s_ea", "s_ln", "s_sm", "s_out"])
        s_i1 = [ec(nc.semaphore(f"s_i1_{q}")) for q in range(NQCH)]
        s_gb1 = [ec(nc.semaphore(f"s_gb1_{b}")) for b in range(RING1)]
        s_zs = [ec(nc.semaphore(f"s_zs_{b}")) for b in range(4)]
        s_zp = ec(nc.semaphore("s_zp"))
        s_gb2 = [ec(nc.semaphore(f"s_gb2_{b}")) for b in range(RING2)]

        # global tile order: tiles listed group by group (= 0..NT-1)
        tile_group = {}
        for gi, g in enumerate(groups):
            for t in g:
                tile_group[t] = gi

        # ---------------- sync engine: uploads + stores ----------------
        @block.sync
        def _(se: bass.BassEngine):
            for q in range(NQCH):
                se.dma_start(idx_sb[:, q_off[q]:q_off[q + 1]],
                             idx1p[:, q_off[q]:q_off[q + 1]]).then_inc(s_i1[q], 16)
            se.dma_start(w1t_sb[:], w1tp[:]).then_inc(s_c, 16)
            se.dma_start(b1c_sb[:], b1cp[:]).then_inc(s_c, 16)
            se.dma_start(w2a_sb[:], w2ap[:]).then_inc(s_c, 16)
            se.dma_start(gpb_sb[:], gpbp[:]).then_inc(s_c, 16)
            se.dma_start(dvc2_sb[:], dvc2p[:]).then_inc(s_c, 16)
            se.dma_start(dvc3_sb[:], dvc3p[:]).then_inc(s_c, 16)
            se.dma_start(gb2_sb[:], gb2p[:].rearrange("p (t f) -> p t f", f=NCLASS)).then_inc(s_c, 16)
            se.dma_start(zro_sb[:], zrop[:]).then_inc(s_c, 16)
            # z pad row (zro upload must land first)
            se.wait_ge(s_c, 128)
            se.dma_start(z_own[CPN:CPN + 1, :], zro_sb[:]).then_inc(s_zp, 16)
            # z stores, one per tile
            for t in range(NT):
                se.wait_ge(s_zq, t + 1)
                se.dma_start(z_own[128 * t:128 * (t + 1), :],
                             zsb[:, t % 4, :]).then_inc(s_zs[t % 4], 16)
            if do_ag:
                # idx2 upload (gathers all consumed by then)
                se.wait_ge(s_red1, NCALLS)
                se.dma_start(idx_sb[:], idx2p[:]).then_inc(s_i2, 16)
            if do_l2:
                # output stores, one per group
                done = 0
                for gi, g in enumerate(groups):
                    done += len(g)
                    se.wait_ge(s_sm, done)
                    t0, t1 = g[0], g[-1] + 1
                    dst_ap = outp[t0 * 128:t1 * 128, :].rearrange("(k p) f -> p k f", p=128)
                    se.dma_start(dst_ap, ob[:, t0:t1, :]).then_inc(s_out, 16)
                se.wait_ge(s_out, 16 * NG)
            elif do_ag:
                se.wait_ge(s_exp, 16 * 8 * NCLS)
            else:
                se.wait_ge(s_zp, 16)
                for sl in range(4):
                    se.wait_ge(s_zs[sl], 16 * ((NT - sl + 3) // 4))

        # ---------------- gpsimd: gathers + collectives ----------------
        @block.gpsimd
        def _(g: bass.BassGpSimd):
            g.load_library(mlp)
            # AG chunk issue positions (call index) and boundary tiles
            bounds = [R[ch + 1] // 128 for ch in range(NCHUNK)]
            # earliest call index at which tile `b-1`'s group is fully issued
            def _call_of(b):
                for gi, g in enumerate(groups):
                    if b - 1 in g:
                        return (gi + 1) * NCLS
                return NCALLS
            ag_pos = [min(_call_of(bounds[ch]) + AG_MARGIN, NCALLS)
                      for ch in range(NCHUNK)]

            def _ag(ch):
                g.wait_ge(s_zp, 16)
                M = min(NT, bounds[ch] + 4)
                for sl in range(4):
                    need = (M - sl + 3) // 4
                    if need > 0:
                        g.wait_ge(s_zs[sl], 16 * need)
                g.collective_compute(
                    "AllGather", mybir.AluOpType.bypass,
                    replica_groups=[list(range(NCORES))],
                    ins=[z_own[R[ch]:R[ch + 1], :].opt()],
                    outs=[z_full[8 * R[ch]:8 * R[ch + 1], :].opt()],
                ).then_inc(s_cc)

            # layer 1 gathers (transpose mode)
            off = 0
            for j, (c, tiles, offs, nsl) in enumerate(calls):
                if do_ag:
                    for ch in range(NCHUNK):
                        if ag_pos[ch] == j:
                            _ag(ch)
                q = next(qq for qq in range(NQCH) if q_calls[qq + 1] > j)
                g.wait_ge(s_i1[q], 16)
                if j >= RING1:
                    g.wait_ge(s_red1, j - RING1 + 1)
                cols = nsl * 128
                g.dma_gather(
                    gbuf1[:, j % RING1, :, 0:cols],
                    xtbl[c * CLS_ROWS:(c + 1) * CLS_ROWS, :],
                    idx_sb[:, off * 8:(off + nsl) * 8],
                    cols, cols, NFEAT,
                    transpose=True,
                    single_packet=False,
                ).then_inc(s_gb1[j % RING1], 16)
                off += nsl
            if do_ag:
                for ch in range(NCHUNK):
                    if ag_pos[ch] >= NCALLS:
                        _ag(ch)
            # expansion: chunk-major so pieces run as their chunk lands
            for ch in (range(NCHUNK) if do_ag else []):
                Lc = R[ch + 1] - R[ch]
                g.wait_ge(s_cc, ch + 1)
                for k in range(NCORES):
                    g.dma_start(
                        ztbl[k * BLK + R[ch]:k * BLK + R[ch] + Lc, 0:NCLASS],
                        z_full[8 * R[ch] + k * Lc:8 * R[ch] + (k + 1) * Lc, :],
                    ).then_inc(s_exp, 16)
            # layer 2 gathers
            if do_l2:
                g.wait_ge(s_i2, 16)
            off = 0
            for j, (c, tiles, offs, nsl) in enumerate(calls if do_l2 else []):
                g.wait_ge(s_exp, 16 * 8 * NCHUNK)
                if j >= RING2:
                    g.wait_ge(s_red2, j - RING2 + 1)
                nidx = nsl * 128
                g.dma_gather(
                    gbuf2[:, j % RING2, 0:nsl, :],
                    ztbl[c * CLS_ROWS:(c + 1) * CLS_ROWS, :],
                    idx_sb[:, off * 8:(off + nsl) * 8],
                    nidx, nidx, F2,
                    single_packet=False,
                ).then_inc(s_gb2[j % RING2], 16)
                off += nsl

        # ---------------- tensor engine ----------------
        @block.tensor
        def _(te):
            te.wait_ge(s_c, 128)
            for t in range(NT):
                te.wait_ge(s_u, t + 1)
                if t >= 2:
                    te.wait_ge(s_v1, t - 1)      # ph ring slot free
                te.matmul(ph[:, t % 2, 0:128], w1t_sb[:, :], u_sb[:, t % U_RING, :],
                          start=True, stop=False)
                te.matmul(ph[:, t % 2, 0:128], b1c_sb[0:1, :],
                          gpb_sb[0:1, 128 * t:128 * (t + 1)],
                          start=False, stop=True).then_inc(s_h)
                if t >= 1:
                    te.wait_ge(s_v1, t)
                    if t >= 3:
                        te.wait_ge(s_zq, t - 2)  # pz ring slot free
                    te.matmul(pz[:, (t - 1) % 2, 0:NCLASS],
                              vT[:, (t - 1) % 2, :], w2a_sb[:, :],
                              start=True, stop=True).then_inc(s_mm2)
            te.wait_ge(s_v1, NT)
            te.wait_ge(s_zq, NT - 2)
            te.matmul(pz[:, (NT - 1) % 2, 0:NCLASS],
                      vT[:, (NT - 1) % 2, :], w2a_sb[:, :],
                      start=True, stop=True).then_inc(s_mm2)

        # ---------------- vector engine ----------------
        @block.vector
        def _(v: bass.BassVectorEngine):
            v.wait_ge(s_c, 128)

            # ---- layer 1 ----
            for gi, g in enumerate(groups):
                for c in range(NCLS):
                    j = gi * NCLS + c
                    (_, tiles, offs, nsl) = calls[j]
                    v.wait_ge(s_gb1[j % RING1], 16 * (j // RING1 + 1))
                    with nc.allow_low_precision(reason="bf16 partial aggregates"):
                        for ti, t in enumerate(tiles):
                            K = int(kmax[t, c])
                            seg = gbuf1[:, j % RING1, 0,
                                        128 * offs[ti]:128 * (offs[ti] + K)]
                            seg = seg.rearrange("p (d k) -> p d k", k=K)
                            v.tensor_reduce(up4[:, ti, c, :], seg,
                                            axis=mybir.AxisListType.X, op=OP.add)
                    v.nop().then_inc(s_red1, 1)
                for ti, t in enumerate(g):
                    if t >= U_RING:
                        v.wait_ge(s_h, t - U_RING + 1)   # u_sb ring slot free
                    v.tensor_add(tA[:, :], up4[:, ti, 0, :], up4[:, ti, 1, :])
                    v.tensor_add(tB[:, :], up4[:, ti, 2, :], up4[:, ti, 3, :])
                    v.tensor_add(u_sb[:, t % U_RING, :], tA[:, :], tB[:, :]).then_inc(s_u)
            # ---- layer 2 ----
            def _final(gi):
                for t in groups[gi]:
                    v.wait_ge(s_ln, gi + 1)
                    v.tensor_scalar(out=ob[:, t, :], in0=ob[:, t, :],
                                    scalar1=lnl[:, t:t + 1], scalar2=None,
                                    op0=OP.subtract).then_inc(s_sm)

            for gi, g in enumerate(groups if do_l2 else []):
                for c in range(NCLS):
                    j = gi * NCLS + c
                    (_, tiles, offs, nsl) = calls[j]
                    v.wait_ge(s_gb2[j % RING2], 16 * (j // RING2 + 1))
                    for ti, t in enumerate(tiles):
                        K = int(kmax[t, c])
                        seg = gbuf2[:, j % RING2, offs[ti]:offs[ti] + K, 0:NCLASS]
                        seg = seg.rearrange("p k f -> p f k")
                        v.tensor_reduce(o4[:, ti, c, :], seg,
                                        axis=mybir.AxisListType.X, op=OP.add)
                    v.nop().then_inc(s_red2, 1)
                for ti, t in enumerate(g):
                    v.tensor_add(oA[:, :], o4[:, ti, 0, :], o4[:, ti, 1, :])
                    v.tensor_add(oB[:, :], o4[:, ti, 2, :], o4[:, ti, 3, :])
                    v.tensor_add(oA[:, :], oA[:, :], oB[:, :])
                    v.scalar_tensor_tensor(
                        out=ob[:, t, :], in0=oA[:, :],
                        scalar=dvc3_sb[:, t:t + 1], in1=gb2_sb[:, t, :],
                        op0=OP.mult, op1=OP.add).then_inc(s_ob)
                if gi >= 1:
                    _final(gi - 1)
            if do_l2:
                _final(NG - 1)

        # ---------------- scalar engine (Act) ----------------
        @block.scalar
        def _(sc):
            sc.wait_ge(s_c, 128)
            # layer 1: relu + z quantize, software-pipelined per tile
            def _zq(t):
                sc.wait_ge(s_mm2, t + 1)
                if t >= 4:
                    sc.wait_ge(s_zs[t % 4], 16 * ((t - 4) // 4 + 1))  # slot free
                sc.activation(zsb[:, t % 4, :], pz[:, t % 2, 0:NCLASS],
                              AF.Copy, scale=dvc2_sb[:, t:t + 1]).then_inc(s_zq)

            for t in range(NT):
                sc.wait_ge(s_h, t + 1)
                if t >= 2:
                    sc.wait_ge(s_mm2, t - 1)      # vT ring slot free
                sc.activation(vT[:, t % 2, :], ph[:, t % 2, 0:128],
                              AF.Relu).then_inc(s_v1)
                if t >= 1:
                    _zq(t - 1)
            _zq(NT - 1)
            # layer 2: exp accumulate + per-group Ln
            nea = 0
            for gi, g in enumerate(groups if do_l2 else []):
                for t in g[:-1]:
                    sc.wait_ge(s_ob, t + 1)
                    sc.activation(tmp40[:], ob[:, t, :], AF.Exp,
                                  accum_out=lse[:, t:t + 1])
                t = g[-1]
                sc.wait_ge(s_ob, t + 1)
                sc.activation(tmp40[:], ob[:, t, :], AF.Exp,
                              accum_out=lse[:, t:t + 1]).then_inc(s_ea)
                nea += 1
                sc.wait_ge(s_ea, nea)
                t0, t1 = g[0], g[-1] + 1
                sc.activation(lnl[:, t0:t1], lse[:, t0:t1], AF.Ln).then_inc(s_ln)

    nc.compile()
    return nc


_LAST_NC = None


def kernel(x, W1, b1, W2, b2, edge_index):
    global _LAST_NC
    from concourse.bass_utils import run_bass_kernel_spmd
    import ml_dtypes

    x = np.asarray(x)
    W1 = np.asarray(W1); b1 = np.asarray(b1)
    W2 = np.asarray(W2); b2 = np.asarray(b2)
    edge_index = np.asarray(edge_index)

    meta, streams1, streams2 = _host_prep(edge_index)
    calls = meta["calls"]
    groups = meta["groups"]
    kmax = meta["kmax"]
    nc = _build_program(calls, groups, kmax)
    _LAST_NC = nc

    dinv = meta["dinv"]
    gprime = meta["gprime"]
    blocks = meta["blocks"]

    # x table: row k*BLK+pos = dinv[n] * x[n] (bf16); pad rows zero
    xtbl = np.zeros((TBL, NFEAT), ml_dtypes.bfloat16)
    for k in range(NCORES):
        blk = blocks[k]
        real = np.flatnonzero(blk >= 0)
        nn = blk[real]
        xtbl[k * BLK + real] = (dinv[nn][:, None] * x[nn]).astype(ml_dtypes.bfloat16)

    w1t_np = W1.T.astype(ml_dtypes.bfloat16)                 # [128, 64]
    b1c_np = b1.reshape(1, NHID).astype(ml_dtypes.bfloat16)
    w2a_np = W2.T.astype(ml_dtypes.bfloat16)                 # [64, 40]
    zro_np = np.zeros((1, NCLASS), ml_dtypes.float8_e4m3fn)

    in_maps = []
    for k in range(NCORES):
        blk = blocks[k]
        real = blk >= 0
        idxs = np.flatnonzero(real)
        nn = blk[idxs]
        gpb_np = np.zeros(CPN, np.float32)
        gpb_np[idxs] = gprime[nn]
        dvc2_np = np.zeros(CPN, np.float32)
        dvc2_np[idxs] = 64.0 * dinv[nn] * dinv[nn]
        dvc3_np = np.zeros(CPN, np.float32)
        dvc3_np[idxs] = dinv[nn] / 64.0
        gb2_np = np.zeros((CPN, NCLASS), np.float32)
        gb2_np[idxs] = (dinv[nn] * gprime[nn])[:, None] * b2[None, :]
        in_maps.append({
            "xtbl": xtbl,
            "idx1": streams1[k], "idx2": streams2[k],
            "w1t": w1t_np, "b1c": b1c_np, "w2a": w2a_np,
            "gpb": gpb_np.reshape(1, CPN).astype(ml_dtypes.bfloat16),
            "dvc2": dvc2_np.reshape(NT, 128).T.copy(),
            "dvc3": dvc3_np.reshape(NT, 128).T.copy(),
            "gb2": gb2_np.reshape(NT, 128, NCLASS).transpose(1, 0, 2)
                        .reshape(128, NT * NCLASS).astype(ml_dtypes.bfloat16),
            "zro": zro_np,
        })

    res = run_bass_kernel_spmd(nc, in_maps, list(range(NCORES)))

    out = np.empty((N, NCLASS), np.float32)
    for k in range(NCORES):
        blk = blocks[k]
        real = blk >= 0
        out[blk[real]] = res.results[k]["out"][np.flatnonzero(real)]
    return out
